# revision 7
# baseline (speedup 1.0000x reference)
"""Trainium2 Bass kernel for nn_AttentionUnroll (dense_transformer).

Full (unsharded) inputs in, full output out. Internally: head-parallel over
8 NeuronCores (16 heads/core), weights resident in SBUF, 32 recurrent
attention steps with a per-step AllGather of per-head q/k/v activations.

Structure:
  - Wo folded into Wq/Wk/Wv on device at init (W'* = Wo @ W* per head), so
    each step needs one projection pass; recurrent state is the
    pre-output-projection mix y (same arithmetic by associativity).
  - Projections: activation vector stationary, weights moving (PE streams
    weights at 1 col/cycle), 4-way column-tiled.
  - Per-step AllGather of per-head q/kk/vv rows ([16,768] per core);
    attention computed replicated on every core from the gathered [128,768].
  - matmul operands in bf16 (full PE rate); accumulations and the gathered
    activations stay float32.

Slot layout per core: slot s = 4*r + g (g = PE column-group, r = PSUM round).
SBUF/PSUM row for slot s lives at partition 32g, free block r; the
contribution DMA emits rows in (g, r) order, so device slot s holds physical
head 16*core + rowmap(s), rowmap(s) = 4*(s%4) + s//4 — applied host-side.
"""

import sys
import numpy as np

sys.path.insert(0, "/opt/trn_rl_repo")

import concourse.bass as bass
import concourse.bacc as bacc
import concourse.tile as tile
from concourse import mybir
from concourse import bass_utils

N_CORES = 8
H = 128
D = 256
HL = H // N_CORES  # 16 heads per core
N_STEPS = 32
dt = mybir.dt
f32 = dt.float32
bf16 = dt.bfloat16

ROWMAP = np.array([4 * (s % 4) + s // 4 for s in range(HL)])

_CACHE = {}


def build_nc(n_steps=N_STEPS):
    nc = bacc.Bacc("TRN2", target_bir_lowering=False, debug=False,
                   num_devices=N_CORES)

    x_l = nc.dram_tensor("x_l", [HL, D], f32, kind="ExternalInput")
    wq = nc.dram_tensor("wq", [HL, D, D], f32, kind="ExternalInput")
    wk = nc.dram_tensor("wk", [HL, D, D], f32, kind="ExternalInput")
    wv = nc.dram_tensor("wv", [HL, D, D], f32, kind="ExternalInput")
    wo = nc.dram_tensor("wo", [HL, D, D], f32, kind="ExternalInput")
    out_l = nc.dram_tensor("out_l", [HL, D], f32, kind="ExternalOutput")

    ident_dram = nc.inline_tensor(np.eye(128, dtype=np.float32), name="ident")

    with tile.TileContext(nc) as tc:
        with tc.tile_pool(name="persist", bufs=1) as pp, \
             tc.tile_pool(name="wstream", bufs=3) as wsp, \
             tc.tile_pool(name="wotp", bufs=4) as wotp, \
             tc.tile_pool(name="sb", bufs=2) as sb, \
             tc.tile_pool(name="attn", bufs=2) as at, \
             tc.tile_pool(name="pt", bufs=2, space="PSUM") as ppt, \
             tc.tile_pool(name="pqkv", bufs=2, space="PSUM") as pqkv, \
             tc.tile_pool(name="pmisc", bufs=1, space="PSUM") as pms, \
             tc.tile_pool(name="dram", bufs=2, space="DRAM") as dr:

            ident = pp.tile([128, 128], bf16, tag="ident")
            nc.gpsimd.dma_start(ident[:], ident_dram[:])

            pid = nc.vector.partition_id()

            # ---- persistent weights ----
            wf = pp.tile([128, HL * 2 * 768], bf16, tag="wfused")
            wo_sb = pp.tile([128, HL * 2 * 256], bf16, tag="wo")
            nc.gpsimd.dma_start(
                wo_sb[:].rearrange("p (s k e) -> p s k e", s=HL, k=2),
                wo.rearrange("s (k p) e -> p s k e", k=2))

            # ---- init: x0 transpose ----
            x0_sb = sb.tile([HL, D], bf16, tag="x0")
            nc.gpsimd.dma_start(x0_sb[:], x_l[:])
            x0T = pp.tile([128, 32], bf16, tag="x0T")
            for k in range(2):
                t = ppt.tile([128, HL], bf16, tag="pt")
                nc.tensor.transpose(t[:], x0_sb[:, 128 * k:128 * (k + 1)],
                                    ident[0:HL, 0:HL])
                nc.vector.tensor_copy(x0T[:, 16 * k:16 * (k + 1)], t[:])

            # ---- init: W' = Wo @ W* products + step-1 projections ----
            qkv1_sb = sb.tile([128, 4 * 768], f32, tag="qkvsb",
                              name="qkv1_sb")
            rnd_ps = None
            for s in range(HL):
                g, r = s % 4, s // 4
                w1 = wsp.tile([128, 2, 768], bf16, tag="w1")
                nc.gpsimd.dma_start(
                    w1[:, :, 0:256], wq[s].rearrange("(k p) e -> p k e", k=2))
                nc.gpsimd.dma_start(
                    w1[:, :, 256:512], wk[s].rearrange("(k p) e -> p k e", k=2))
                nc.gpsimd.dma_start(
                    w1[:, :, 512:768], wv[s].rearrange("(k p) e -> p k e", k=2))

                # WoT blocks (a = m-chunk, b = d-chunk) = T(Wo block (b, a))
                wot = [[None] * 2 for _ in range(2)]
                for a in range(2):
                    for b in range(2):
                        t = ppt.tile([128, 128], bf16, tag="pt",
                                     name=f"wotT_{s}_{a}_{b}")
                        nc.tensor.transpose(
                            t[:],
                            wo_sb[:, (2 * s + b) * 256 + 128 * a:
                                     (2 * s + b) * 256 + 128 * (a + 1)],
                            ident[:])
                        w = wotp.tile([128, 128], bf16, tag="wot",
                                      name=f"wot_{s}_{a}_{b}")
                        nc.vector.tensor_copy(w[:], t[:])
                        wot[a][b] = w

                # W'[d, e'] = sum_m Wo[d, m] W*[m, e']
                for j in range(2):
                    ps = pqkv.tile([128, 768], f32, tag="qkvp",
                                   name=f"wprod_{s}_{j}")
                    for a in range(2):
                        for (n0, n1) in ((0, 512), (512, 768)):
                            nc.tensor.matmul(
                                ps[:, n0:n1], (wot[a][j][:]),
                                (w1[:, a, n0:n1]),
                                start=(a == 0), stop=(a == 1))
                    dst = wf[:, (2 * s + j) * 768:(2 * s + j + 1) * 768]
                    nc.vector.tensor_copy(dst[0:128, 0:256], ps[:, 0:256])
                    nc.vector.tensor_scalar_mul(dst[0:128, 256:512],
                                                ps[:, 256:512], 0.125)
                    nc.vector.tensor_copy(dst[0:128, 512:768], ps[:, 512:768])

                # step-1 projections (original weights; scores scaled later)
                if g == 0:
                    rnd_ps = pqkv.tile([128, 768], f32, tag="qkvp",
                                       name=f"qkv1_r{r}")
                for k in range(2):
                    for (n0, n1) in ((0, 512), (512, 768)):
                        nc.tensor.matmul(
                            rnd_ps[32 * g:32 * g + 1, n0:n1],
                            (x0T[:, 16 * k + s:16 * k + s + 1]),
                            (w1[:, k, n0:n1]),
                            start=(k == 0), stop=(k == 1),
                            tile_position=(0, 32 * g))
                if g == 3:
                    nc.vector.tensor_copy(
                        qkv1_sb[:, 768 * r:768 * (r + 1)], rnd_ps[:])

            # ---- steps ----
            yn_sb = None
            for t in range(1, n_steps + 1):
                step1 = (t == 1)
                if step1:
                    qkv_sb = qkv1_sb
                else:
                    # ynT: transpose state, extract local columns
                    ynT = sb.tile([128, 256], bf16, tag="ynT",
                                  name=f"ynT_{t}")
                    for k in range(2):
                        tt = ppt.tile([128, 128], bf16, tag="pt",
                                      name=f"ynTt_{t}_{k}")
                        nc.tensor.transpose(
                            tt[:], yn_sb[:, 128 * k:128 * (k + 1)], ident[:])
                        nc.vector.tensor_copy(ynT[:, 128 * k:128 * (k + 1)],
                                              tt[:])
                    ynTl = sb.tile([128, 32], bf16, tag="ynTl",
                                   name=f"ynTl_{t}")
                    nc.vector.tensor_copy(
                        ynTl[:].rearrange("p (k c) -> p k c", k=2),
                        ynT[:].rearrange("p (k c) -> p k c", k=2)
                        [:, :, bass.ts(pid, HL)])
                    qkv_sb = sb.tile([128, 4 * 768], f32, tag="qkvsb",
                                     name=f"qkv_sb_{t}")
                    for rr in range(4):
                        rnd = pqkv.tile([128, 768], f32, tag="qkvp",
                                        name=f"qkv_{t}_{rr}")
                        for gg in range(4):
                            s = 4 * rr + gg
                            for k in range(2):
                                for (n0, n1) in ((0, 512), (512, 768)):
                                    nc.tensor.matmul(
                                        rnd[32 * gg:32 * gg + 1, n0:n1],
                                        (ynTl[:, 16 * k + s:16 * k + s + 1]),
                                        (wf[:, (2 * s + k) * 768 + n0:
                                               (2 * s + k) * 768 + n1]),
                                        start=(k == 0), stop=(k == 1),
                                        tile_position=(0, 32 * gg))
                        nc.vector.tensor_copy(
                            qkv_sb[:, 768 * rr:768 * (rr + 1)], rnd[:])

                # --- contribution -> DRAM (compacts rows), AllGather ---
                cc_in = dr.tile([HL, 768], f32, tag="ccin", name=f"ccin_{t}")
                cc_out = dr.tile([H, 768], f32, tag="ccout",
                                 addr_space="Shared", name=f"ccout_{t}")
                nc.sync.dma_start(
                    cc_in[:].rearrange("(g r) e -> g r e", g=4),
                    qkv_sb[:].rearrange("(g gp) (r e) -> g gp r e",
                                        g=4, r=4)[:, 0, :, :])
                nc.gpsimd.collective_compute(
                    "AllGather", mybir.AluOpType.bypass,
                    replica_groups=[list(range(N_CORES))],
                    ins=[cc_in.opt()], outs=[cc_out.opt()])
                gath = at.tile([128, 768], bf16, tag="gath", name=f"gath_{t}")
                nc.gpsimd.dma_start(gath[:], cc_out[:])

                # --- qT, kkT from gathered rows ---
                qT = at.tile([128, 256], bf16, tag="qT", name=f"qT_{t}")
                kkT = at.tile([128, 256], bf16, tag="kkT", name=f"kkT_{t}")
                for k in range(2):
                    tq = ppt.tile([128, 128], bf16, tag="pt",
                                  name=f"qTt_{t}_{k}")
                    nc.tensor.transpose(tq[:], gath[:, 128 * k:128 * (k + 1)],
                                        ident[:])
                    nc.vector.tensor_copy(qT[:, 128 * k:128 * (k + 1)], tq[:])
                    tk = ppt.tile([128, 128], bf16, tag="pt",
                                  name=f"kkTt_{t}_{k}")
                    nc.tensor.transpose(
                        tk[:], gath[:, 256 + 128 * k:256 + 128 * (k + 1)],
                        ident[:])
                    nc.vector.tensor_copy(kkT[:, 128 * k:128 * (k + 1)], tk[:])

                # --- scores, softmax ---
                S = pms.tile([128, 128], f32, tag="S", name=f"S_{t}")
                for k in range(2):
                    nc.tensor.matmul(S[:], (kkT[:, 128 * k:128 * (k + 1)]),
                                     (qT[:, 128 * k:128 * (k + 1)]),
                                     start=(k == 0), stop=(k == 1))
                negmax = at.tile([128, 1], f32, tag="negmax",
                                 name=f"negmax_{t}")
                nc.vector.tensor_reduce(negmax[:], S[:],
                                        axis=mybir.AxisListType.X,
                                        op=mybir.AluOpType.max, negate=True)
                E = at.tile([128, 128], bf16, tag="E", name=f"E_{t}")
                Z = at.tile([128, 1], f32, tag="Z", name=f"Z_{t}")
                if step1:
                    nms = at.tile([128, 1], f32, tag="nms", name=f"nms_{t}")
                    nc.vector.tensor_scalar_mul(nms[:], negmax[:], 0.125)
                    nc.scalar.activation(E[:], S[:],
                                         mybir.ActivationFunctionType.Exp,
                                         bias=nms[:], scale=0.125,
                                         accum_out=Z[:])
                else:
                    nc.scalar.activation(E[:], S[:],
                                         mybir.ActivationFunctionType.Exp,
                                         bias=negmax[:], scale=1.0,
                                         accum_out=Z[:])
                rinv = at.tile([128, 1], f32, tag="rinv", name=f"rinv_{t}")
                nc.vector.reciprocal(rinv[:], Z[:])

                # --- ET, y, normalize ---
                ET = at.tile([128, 128], bf16, tag="ET", name=f"ET_{t}")
                tt = ppt.tile([128, 128], bf16, tag="pt", name=f"ETt_{t}")
                nc.tensor.transpose(tt[:], E[:], ident[:])
                nc.vector.tensor_copy(ET[:], tt[:])
                y_ps = pms.tile([128, 256], f32, tag="y", name=f"y_{t}")
                nc.tensor.matmul(y_ps[:], (ET[:]), (gath[:, 512:768]),
                                 start=True, stop=True)
                yn_sb = sb.tile([128, 256], bf16, tag="yn", name=f"yn_{t}")
                nc.scalar.activation(yn_sb[:], y_ps[:],
                                     mybir.ActivationFunctionType.Copy,
                                     scale=rinv[:])

            # ---- final output projection: x_out = yn @ Wo (local heads) ----
            ynT = sb.tile([128, 256], bf16, tag="ynT", name="ynT_fin")
            for k in range(2):
                tt = ppt.tile([128, 128], bf16, tag="pt", name=f"ynTtf_{k}")
                nc.tensor.transpose(tt[:], yn_sb[:, 128 * k:128 * (k + 1)],
                                    ident[:])
                nc.vector.tensor_copy(ynT[:, 128 * k:128 * (k + 1)], tt[:])
            ynTl = sb.tile([128, 32], bf16, tag="ynTl", name="ynTl_fin")
            nc.vector.tensor_copy(
                ynTl[:].rearrange("p (k c) -> p k c", k=2),
                ynT[:].rearrange("p (k c) -> p k c", k=2)
                [:, :, bass.ts(pid, HL)])
            xo_sb = sb.tile([128, 1024], f32, tag="xosb", name="xo_sb")
            rnd_ps = None
            for s in range(HL):
                g, r = s % 4, s // 4
                if g == 0:
                    rnd_ps = pqkv.tile([128, 256], f32, tag="qkvp",
                                       name=f"xo_r{r}")
                for k in range(2):
                    nc.tensor.matmul(
                        rnd_ps[32 * g:32 * g + 1, :],
                        (ynTl[:, 16 * k + s:16 * k + s + 1]),
                        (wo_sb[:, (2 * s + k) * 256:(2 * s + k + 1) * 256]),
                        start=(k == 0), stop=(k == 1),
                        tile_position=(0, 32 * g))
                if g == 3:
                    nc.vector.tensor_copy(
                        xo_sb[:, 256 * r:256 * (r + 1)], rnd_ps[:])
            nc.sync.dma_start(
                out_l[:].rearrange("(g r) e -> g r e", g=4),
                xo_sb[:].rearrange("(g gp) (r e) -> g gp r e",
                                   g=4, r=4)[:, 0, :, :])

    nc.compile()
    return nc


def _get_nc(n_steps=N_STEPS):
    if n_steps not in _CACHE:
        _CACHE[n_steps] = build_nc(n_steps)
    return _CACHE[n_steps]


def make_in_maps(x, weight_q, weight_k, weight_v, weight_o):
    x = np.asarray(x).reshape(H, D).astype(np.float32)
    ws = [np.asarray(w, dtype=np.float32)
          for w in (weight_q, weight_k, weight_v, weight_o)]
    in_maps = []
    for c in range(N_CORES):
        idx = HL * c + ROWMAP
        in_maps.append({
            "x_l": np.ascontiguousarray(x[idx]),
            "wq": np.ascontiguousarray(ws[0][idx]),
            "wk": np.ascontiguousarray(ws[1][idx]),
            "wv": np.ascontiguousarray(ws[2][idx]),
            "wo": np.ascontiguousarray(ws[3][idx]),
        })
    return in_maps


def kernel(x, k, v, weight_q, weight_k, weight_v, weight_o, n_steps=N_STEPS):
    del k, v  # dead in the reference
    nc = _get_nc(n_steps)
    in_maps = make_in_maps(x, weight_q, weight_k, weight_v, weight_o)
    res = bass_utils.run_bass_kernel_spmd(nc, in_maps,
                                          core_ids=list(range(N_CORES)))
    out = np.concatenate([res.results[c]["out_l"] for c in range(N_CORES)],
                         axis=0)
    return out.reshape(1, H, 1, D).astype(np.float32)


if __name__ == "__main__":
    rng = np.random.default_rng(0)
    sc = 0.04
    ins = {
        "x": rng.standard_normal((1, H, 1, D)).astype(np.float32) * sc,
        "k": np.zeros((1, H, D), np.float32),
        "v": np.zeros((1, H, D), np.float32),
        "weight_q": rng.standard_normal((H, D, D)).astype(np.float32) * sc,
        "weight_k": rng.standard_normal((H, D, D)).astype(np.float32) * sc,
        "weight_v": rng.standard_normal((H, D, D)).astype(np.float32) * sc,
        "weight_o": rng.standard_normal((H, D, D)).astype(np.float32) * sc,
    }
    got = kernel(n_steps=2, **ins)
    print("kernel out", got.shape, got.flatten()[:4])

    # numpy reference (unfused arithmetic)
    def np_ref(x, wq, wk, wv, wo, n_steps):
        xc = x.reshape(H, D).astype(np.float64)
        for _ in range(n_steps):
            q = np.einsum('hd,hde->he', xc, wq)
            kk = np.einsum('hd,hde->he', xc, wk)
            vv = np.einsum('hd,hde->he', xc, wv)
            s = (kk @ q.T) * 0.125
            a = np.exp(s - s.max(axis=1, keepdims=True))
            a /= a.sum(axis=1, keepdims=True)
            y = a @ vv
            xc = np.einsum('hd,hde->he', y, wo)
        return xc

    want = np_ref(ins["x"], ins["weight_q"].astype(np.float64),
                  ins["weight_k"].astype(np.float64),
                  ins["weight_v"].astype(np.float64),
                  ins["weight_o"].astype(np.float64), 2)
    got2 = got.reshape(H, D)
    err = np.abs(got2 - want).max() / np.abs(want).max()
    print("rel err vs np:", err)
